# revision 3
# baseline (speedup 1.0000x reference)
"""GAT message-passing kernel for 8 Trainium2 NeuronCores.

Strategy (edge-parallel by dst-range, no collectives):
  - Host: sort edges by dst; core c owns dst nodes [c*12500, (c+1)*12500).
    Within a core, dst nodes are tiled 128 at a time (98 tiles); each tile's
    edges are split into chunks of 128 (padded; chunk count per tile = max
    over cores so the SPMD instruction stream is identical on all cores).
  - Device, per chunk of 128 edges (edges on partitions):
      hk_g   [128e, 64]  <- indirect DMA gather of hk[src]
      hk_gT  [64, 128e]  <- PE transpose
      S.T    [128e,128d] <- matmul(lhsT=hk_gT, rhs=huT_tile)   (scores, fp32)
      expS   [128e,128d] <- ACT exp -> bf16 (no max-subtraction needed:
                            |score| <~ 45 so exp stays finite in fp32)
      P.T    [128e,128d] <- expS * onehot(local_dst == iota)   (bf16)
      rst    [128d, 65]  += P.T^T @ [hk_g_bf16 | 1]            (PSUM accum)
    Per dst-tile epilogue: alpha-normalize by column 64 (the segment sum),
    PE transpose, FC matmul with host-prepared [W^T; b] (bias via ones row),
    ReLU, DMA out.
"""
import sys

for p in ("/opt/trn_rl_repo",):
    if p not in sys.path:
        sys.path.insert(0, p)

import numpy as np
import concourse.bass as bass
import concourse.tile as tile
from concourse import mybir, bacc
from concourse.bass_utils import run_bass_kernel_spmd
from concourse.masks import make_identity

f32 = mybir.dt.float32
bf16 = mybir.dt.bfloat16
i32 = mybir.dt.int32

N_CORES = 8
P = 128


def build_gat_kernel(n_nodes_core, n_tiles, g_list, nk_rows, d_feat, d_out):
    """Build the per-core SPMD kernel. g_list[t] = #128-edge chunks in tile t."""
    sum_g = sum(g_list)
    pad_nodes = n_tiles * P
    nc = bacc.Bacc("TRN2", target_bir_lowering=False, debug=False,
                   num_devices=N_CORES)
    hk = nc.dram_tensor("hk", [nk_rows, d_feat], f32, kind="ExternalInput")
    hut = nc.dram_tensor("hut", [d_feat, pad_nodes], f32, kind="ExternalInput")
    srcidx = nc.dram_tensor("srcidx", [P, sum_g], i32, kind="ExternalInput")
    ldst = nc.dram_tensor("ldst", [P, sum_g], f32, kind="ExternalInput")
    wt_aug = nc.dram_tensor("wt_aug", [d_feat + 1, d_out], f32,
                            kind="ExternalInput")
    iota_row = nc.dram_tensor("iota_row", [P, P], f32, kind="ExternalInput")
    y = nc.dram_tensor("y", [n_nodes_core, d_out], f32, kind="ExternalOutput")

    with tile.TileContext(nc) as tc:
        with (
            tc.tile_pool(name="const", bufs=1) as cpool,
            tc.tile_pool(name="work", bufs=4) as pool,
            tc.tile_pool(name="epi", bufs=2) as epool,
            tc.tile_pool(name="ps_st", bufs=2, space="PSUM") as ps_st,
            tc.tile_pool(name="ps_tr", bufs=2, space="PSUM") as ps_tr,
            tc.tile_pool(name="ps_rst", bufs=2, space="PSUM") as ps_rst,
            tc.tile_pool(name="ps_epi", bufs=1, space="PSUM") as ps_epi,
        ):
            ident = cpool.tile([P, P], f32)
            make_identity(nc, ident[:])
            wt_sb = cpool.tile([d_feat + 1, d_out], f32)
            nc.sync.dma_start(wt_sb[:], wt_aug.ap())
            iota_sb = cpool.tile([P, P], f32)
            nc.sync.dma_start(iota_sb[:], iota_row.ap())
            # resident hu^T slice [64, pad_nodes]
            hut_sb = cpool.tile([d_feat, pad_nodes], f32)
            nc.sync.dma_start(hut_sb[:], hut.ap())
            # resident per-chunk metadata
            sidx_sb = cpool.tile([P, sum_g], i32)
            nc.sync.dma_start(sidx_sb[:], srcidx.ap())
            ldst_sb = cpool.tile([P, sum_g], f32)
            nc.sync.dma_start(ldst_sb[:], ldst.ap())

            goff = 0
            for t in range(n_tiles):
                gt = g_list[t]
                hut_t = hut_sb[:, t * P:(t + 1) * P]
                rst_ps = ps_rst.tile([P, d_feat + 1], f32, tag="rst")
                for g in range(gt):
                    col = goff + g
                    hk_g = pool.tile([P, d_feat], f32, tag="hk_g")
                    nc.gpsimd.indirect_dma_start(
                        out=hk_g[:], out_offset=None, in_=hk.ap(),
                        in_offset=bass.IndirectOffsetOnAxis(
                            ap=sidx_sb[:, col:col + 1], axis=0))
                    hkT_ps = ps_tr.tile([d_feat, P], f32, tag="hkT")
                    nc.tensor.transpose(out=hkT_ps[:], in_=hk_g[:],
                                        identity=ident[:])
                    hkT = pool.tile([d_feat, P], f32, tag="hkT_sb")
                    nc.vector.tensor_copy(out=hkT[:], in_=hkT_ps[:])

                    st_ps = ps_st.tile([P, P], f32, tag="st")
                    nc.tensor.matmul(out=st_ps[:], lhsT=hkT[:], rhs=hut_t,
                                     start=True, stop=True)
                    exps = pool.tile([P, P], bf16, tag="exps")
                    nc.scalar.activation(exps[:], st_ps[:],
                                         mybir.ActivationFunctionType.Exp)
                    onehot = pool.tile([P, P], bf16, tag="onehot")
                    nc.vector.tensor_tensor(
                        out=onehot[:],
                        in0=ldst_sb[:, col:col + 1].to_broadcast([P, P]),
                        in1=iota_sb[:],
                        op=mybir.AluOpType.is_equal)
                    pt = pool.tile([P, P], bf16, tag="pt")
                    nc.vector.tensor_tensor(out=pt[:], in0=exps[:],
                                            in1=onehot[:],
                                            op=mybir.AluOpType.mult)
                    vals = pool.tile([P, d_feat + 1], bf16, tag="vals")
                    nc.vector.tensor_copy(out=vals[:, 0:d_feat], in_=hk_g[:])
                    nc.vector.memset(vals[:, d_feat:d_feat + 1], 1.0)
                    nc.tensor.matmul(out=rst_ps[:], lhsT=pt[:], rhs=vals[:],
                                     start=(g == 0), stop=(g == gt - 1))
                goff += gt

                # epilogue: normalize, transpose, FC, relu, store
                denom = epool.tile([P, 1], f32, tag="denom")
                nc.vector.tensor_scalar_add(denom[:],
                                            rst_ps[:, d_feat:d_feat + 1], 1e-30)
                recip = epool.tile([P, 1], f32, tag="recip")
                nc.vector.reciprocal(recip[:], denom[:])
                rst_sb = epool.tile([P, d_feat + 1], f32, tag="rst_sb")
                nc.vector.tensor_scalar_mul(rst_sb[:, 0:d_feat],
                                            rst_ps[:, 0:d_feat], recip[:])
                nc.vector.memset(rst_sb[:, d_feat:d_feat + 1], 1.0)

                rstT_ps = ps_epi.tile([d_feat + 1, P], f32, tag="rstT")
                nc.tensor.transpose(out=rstT_ps[:], in_=rst_sb[:],
                                    identity=ident[:])
                rstT = epool.tile([d_feat + 1, P], f32, tag="rstT_sb")
                nc.vector.tensor_copy(out=rstT[:], in_=rstT_ps[:])

                out_ps = ps_epi.tile([P, d_out], f32, tag="out_ps")
                nc.tensor.matmul(out=out_ps[:], lhsT=rstT[:], rhs=wt_sb[:],
                                 start=True, stop=True)
                out_sb = epool.tile([P, d_out], f32, tag="out_sb")
                nc.scalar.activation(out_sb[:], out_ps[:],
                                     mybir.ActivationFunctionType.Relu)
                rows = min(P, n_nodes_core - t * P)
                nc.sync.dma_start(y.ap()[t * P:t * P + rows], out_sb[:rows])
    nc.compile()
    return nc


def prep_inputs(hk, hu, W, b, src, dst, n_cores=N_CORES):
    """Host-side sharding prep. Returns (per-core in_maps, g_list, meta)."""
    n_nodes, d_feat = hk.shape
    d_out = W.shape[0]
    npc = n_nodes // n_cores          # nodes per core
    n_tiles = (npc + P - 1) // P
    pad_nodes = n_tiles * P

    src = np.ascontiguousarray(src.astype(np.int32))
    dst = np.ascontiguousarray(dst.astype(np.int32))
    order = np.argsort(dst, kind="stable")
    dst_s = dst[order]
    src_s = src[order]

    # edge count per (core, tile): tiles are 128-node blocks LOCAL to each
    # core's [c*npc, (c+1)*npc) range (npc need not be a multiple of 128).
    core_of = dst_s // npc
    local_tile = (dst_s - core_of * npc) // P
    flat = core_of * n_tiles + local_tile
    counts = np.bincount(flat, minlength=n_cores * n_tiles)
    counts = counts.reshape(n_cores, n_tiles)
    g_list = np.maximum(1, (counts.max(axis=0) + P - 1) // P).astype(int).tolist()
    sum_g = int(sum(g_list))

    # dst_s is sorted, and flat is non-decreasing along it
    starts = np.zeros(n_cores * n_tiles + 1, np.int64)
    np.cumsum(counts.reshape(-1), out=starts[1:])

    wt_aug = np.concatenate([W.T, b[None, :]], axis=0).astype(np.float32)
    iota_row = np.tile(np.arange(P, dtype=np.float32), (P, 1))
    hk = np.ascontiguousarray(hk, np.float32)

    in_maps = []
    goffs = np.concatenate([[0], np.cumsum(g_list)]).astype(int)
    for c in range(n_cores):
        srcidx = np.zeros((P, sum_g), np.int32)
        ldst = np.full((P, sum_g), 999.0, np.float32)
        for t in range(n_tiles):
            gtile = c * n_tiles + t
            s, e = starts[gtile], starts[gtile + 1]
            cnt = e - s
            if cnt == 0:
                continue
            go = goffs[t]
            j = np.arange(cnt)
            pp = j % P
            gg = j // P
            srcidx[pp, go + gg] = src_s[s:e]
            ldst[pp, go + gg] = (dst_s[s:e] - (c * npc + t * P)).astype(np.float32)
        hut = np.zeros((d_feat, pad_nodes), np.float32)
        hut[:, :npc] = hu[c * npc:(c + 1) * npc].T
        in_maps.append({
            "hk": hk, "hut": hut, "srcidx": srcidx, "ldst": ldst,
            "wt_aug": wt_aug, "iota_row": iota_row,
        })
    meta = dict(npc=npc, n_tiles=n_tiles, n_nodes=n_nodes, d_feat=d_feat,
                d_out=d_out)
    return in_maps, g_list, meta


_KERNEL_CACHE = {}


def run_gat(hk, hu, W, b, src, dst, n_cores=N_CORES):
    in_maps, g_list, meta = prep_inputs(hk, hu, W, b, src, dst, n_cores)
    key = (tuple(g_list), meta["npc"], meta["d_feat"], meta["d_out"],
           hk.shape[0])
    if key not in _KERNEL_CACHE:
        _KERNEL_CACHE[key] = build_gat_kernel(
            meta["npc"], meta["n_tiles"], g_list, hk.shape[0],
            meta["d_feat"], meta["d_out"])
    nc = _KERNEL_CACHE[key]
    res = run_bass_kernel_spmd(nc, in_maps, core_ids=list(range(n_cores)))
    out = np.concatenate([res.results[c]["y"] for c in range(n_cores)], axis=0)
    return np.ascontiguousarray(out, np.float32)


def kernel(hk, hu, W, b, src, dst):
    hk = np.asarray(hk, np.float32)
    hu = np.asarray(hu, np.float32)
    W = np.asarray(W, np.float32)
    b = np.asarray(b, np.float32)
    src = np.asarray(src)
    dst = np.asarray(dst)
    return run_gat(hk, hu, W, b, src, dst)
